# revision 57
# baseline (speedup 1.0000x reference)
"""DeepSeek sparse attention (single-query, MQA low-rank KV) on 8 trn2 cores.

Strategy (data-parallel: batch b -> core b), algebraically folded so the
device never materializes K_down/V_down for unselected tokens:

  Launch 1 (device): noisy indexer scores for all S via a folded matvec
      s[t] = x[t,:] . v,   v = Wkv_down[:, :L] @ q_idx   (host, f32)
      This skips the reference's per-element fp8 quantization of K_down
      (can't be folded); x and v are streamed as fp8e4 and contracted on
      the PE with DoubleRow fp8 matmuls. DMA-bound: streams x.T fp8
      (16.8 MB/core) at ~360 GB/s.
  Host: top-k certain/band split. Measured worst-case noisy-vs-exact rank
      displacement on these inputs is 358; MARGIN=768 gives >2x slack.
      Band (2*MARGIN tokens) rescored bit-exactly vs the reference via
      jax-CPU fp8-emulated slice gemm; union with certain = exact top-k set.
  Launch 2 (device): attention over the k selected tokens with every
      projection folded through x_sel (all bf16 on the PE):
        logits^T[k,h] = x_sel[k] . qhd[h],  qhd = (q_h @ Wk_up_h^T) @ Wkv_down_K^T
          -- computed directly in the k-partitioned layout; exp() of it IS
          attn^T (unnormalized; per-head logit constants drop out of softmax,
          1/den is applied on host since the V path is linear in attn).
        m^T  = x_sel^T @ attn^T      [D, H]  (x_sel rebuilt on-device from
                                              x_sel^T by PE transposes)
        mv^T = Wv_down_V^T-contraction of m^T          [L, H]
        o^T[dh,h] = Wv_up_h^T @ mv[:,h]                [DH, H]
      Exports o^T and den; host normalizes (o / den).
  Launch 3 (device): out = o_all @ Wout, tensor-parallel: core c holds
      Wout[:, c-block] (1/8th) and all 8 batches' o (32 KiB); host
      concatenates the column blocks and adds bout' (all V/out biases
      folded via sum(attn)=1 into a single out-bias).

Shapes hardcoded: B=8, S=8192, D=2048, H=16, dh=128, L=512, k=2048.
"""
import numpy as np
import ml_dtypes

import concourse.bacc as bacc
import concourse.tile as tile
import concourse.mybir as mybir
from concourse import masks
from concourse.bass_utils import run_bass_kernel_spmd

BF16 = ml_dtypes.bfloat16
F8 = ml_dtypes.float8_e4m3fn
dt = mybir.dt

B, S, D = 8, 8192, 2048
H, DH, L = 16, 128, 512
TOPK = 2048
MARGIN = 768
NCORES = 8
ND = D // 128            # 16 d-chunks
NL = L // 128            # 4 l-chunks
NK = TOPK // 128         # 16 k-chunks
RSQ = float(1.0 / np.sqrt(np.float32(DH)))

_STATE = {}
LAST_EXEC = {}


# ---------------------------------------------------------------- launch 1
def _build_l1():
    """Noisy scores for all S tokens: scores[s] = sum_d x[s,d] * v[d].

    fp8 DoubleRow matvec: stationary v pairs [128,2,128] (col 0 real,
    rest zero-pad for the ISA check), moving x pairs [128,2,512] ->
    out row 0 of [128,512], accumulating 8 pairs (K=2048) in PSUM.
    """
    nc = bacc.Bacc("TRN2", target_bir_lowering=False, debug=False,
                   num_devices=NCORES)
    x8 = nc.dram_tensor("x8", [D, S], dt.float8e4, kind="ExternalInput").ap()
    # v zero-padded to a 32-wide stationary (column 0 = v): the walrus ISA
    # check rejects DoubleRow Ldweights with M<32, and the modeled matmul
    # cost only depends on the moving size anyway.
    v8 = nc.dram_tensor("v8", [128, ND * 32], dt.float8e4,
                        kind="ExternalInput").ap()
    scores = nc.dram_tensor("scores", [1, S], dt.float32,
                            kind="ExternalOutput").ap()

    # Uniform octants: fine enough that the tail compute after the final
    # transfer is small, coarse enough that HWDGE (625ns/DMA, 73 DMAs =
    # 45.7us) stays just under the 46.6us transfer stream. Coarser early
    # segments were tried and regressed (+0.9us).
    SEGS = [(i * 1024, 1024) for i in range(8)]
    NP = ND // 2               # 8 d-chunk pairs

    with tile.TileContext(nc) as tc:
        with (
            tc.tile_pool(name="wpool", bufs=1) as wpool,
            tc.tile_pool(name="xpool", bufs=2) as xpool,
            tc.tile_pool(name="pspool", bufs=2, space="PSUM") as pspool,
        ):
            vsb = wpool.tile([128, ND, 32], dt.float8e4)
            nc.sync.dma_start(
                vsb[:], v8.rearrange("p (c m) -> p c m", m=32))
            scr = wpool.tile([1, S], dt.float32)

            for si, (q0, qw) in enumerate(SEGS):
                # fixed max-width tiles sliced to the segment width so the
                # two widths share one pool slot (PSUM budget: 2x4 banks)
                xqf = xpool.tile([128, ND, 2048], dt.float8e4, tag="xq")
                xq = xqf[:, :, :qw]
                for p in range(NP):
                    # one batched DMA per chunk pair = DR matmul granularity
                    nc.sync.dma_start(
                        xq[:, 2 * p:2 * p + 2, :],
                        x8[2 * p * 128:(2 * p + 2) * 128,
                           q0:q0 + qw].rearrange("(c p) w -> p c w", p=128))
                psf = pspool.tile([32, 2048], dt.float32, tag="ps")
                ps = psf[:, :qw]
                for p in range(NP):
                    for nb in range(qw // 512):
                        nc.tensor.matmul(
                            ps[:, nb * 512:(nb + 1) * 512],
                            vsb[:, 2 * p:2 * p + 2, :],
                            xq[:, 2 * p:2 * p + 2, nb * 512:(nb + 1) * 512],
                            start=(p == 0), stop=(p == NP - 1),
                            perf_mode=mybir.MatmulPerfMode.DoubleRow)
                # per-segment write-out; the PSUM->SBUF copies alternate
                # DVE/Activation (GPSIMD cannot access PSUM) and the DMA
                # rides the Activation queue so the x8 load stream on SP
                # is never head-of-line blocked (except the last segment,
                # where SP is idle and fastest).
                cps = [nc.vector.tensor_copy, nc.scalar.copy,
                       nc.vector.tensor_copy, nc.scalar.copy]
                for nb in range(qw // 512):
                    cps[nb](scr[:, q0 + nb * 512:q0 + (nb + 1) * 512],
                            ps[0:1, nb * 512:(nb + 1) * 512])
                (nc.sync if si == len(SEGS) - 1 else nc.scalar).dma_start(
                    scores[:, q0:q0 + qw], scr[:, q0:q0 + qw])
    nc.compile()
    return nc


# ---------------------------------------------------------------- launch 2
def _build_l2():
    nc = bacc.Bacc("TRN2", target_bir_lowering=False, debug=False,
                   num_devices=NCORES)
    bf = dt.bfloat16
    xselT = nc.dram_tensor("xselT", [D, TOPK], bf, kind="ExternalInput").ap()
    # qhd pre-packed on host to the SBUF layout [p, (c h)] so the DMA has
    # 512B-contiguous descriptors (a [D, H] layout would move 32B lines)
    qhdP = nc.dram_tensor("qhdP", [128, ND * H], bf, kind="ExternalInput").ap()
    wvd = nc.dram_tensor("wvd", [D, L], bf, kind="ExternalInput").ap()
    wvup = nc.dram_tensor("wvup", [L, D], bf, kind="ExternalInput").ap()
    oTo = nc.dram_tensor("oTo", [128, H], dt.float32,
                         kind="ExternalOutput").ap()
    deno = nc.dram_tensor("deno", [1, H], dt.float32,
                          kind="ExternalOutput").ap()

    with tile.TileContext(nc) as tc:
        with tc.tile_pool(name="top", bufs=1) as top:
            # ---- SBUF residents (per-partition KiB in comments)
            xtp = top.tile([128, ND, TOPK], bf)    # x_sel^T  64K
            xsp = top.tile([128, NK, D], bf)       # x_sel    64K
            wvdt = top.tile([128, ND, L], bf)      # Wv_down  16K
            wvut = top.tile([128, NL, D], bf)      # Wv_up    16K
            qhdt = top.tile([128, ND, H], bf)
            identf = top.tile([128, 128], dt.float32)
            identb = top.tile([128, 128], bf)
            onesb = top.tile([128, 1], bf)
            attnT = top.tile([128, NK, H], bf)     # exp(logits^T), unnorm.
            mT = top.tile([128, ND, H], bf)
            mvT = top.tile([128, NL, H], bf)
            oT = top.tile([128, H], dt.float32)
            den1 = top.tile([1, H], dt.float32)

            # ---- DMAs (one serialized device — order = packing): xtp
            # chunks first so the transpose pipeline tracks the stream;
            # wvd before wvut so the last-landing tensor feeds the
            # shortest dependent chain (o^T only).
            nc.sync.dma_start(
                qhdt[:], qhdP.rearrange("p (c h) -> p c h", h=H))
            for c in range(ND):
                nc.sync.dma_start(xtp[:, c, :], xselT[c * 128:(c + 1) * 128, :])
            # weights split 4-way so the dependent matmul chains (mv^T per
            # d-group, o^T per l-chunk) track the stream instead of waiting
            # for a monolithic transfer to land
            for g in range(4):
                nc.sync.dma_start(
                    wvdt[:, g * 4:(g + 1) * 4, :],
                    wvd[g * 4 * 128:(g + 1) * 4 * 128, :].rearrange(
                        "(c p) l -> p c l", p=128))
            for lc in range(NL):
                nc.sync.dma_start(
                    wvut[:, lc:lc + 1, :],
                    wvup[lc * 128:(lc + 1) * 128, :].rearrange(
                        "(c p) d -> p c d", p=128))

            masks.make_identity(nc, identf[:])
            nc.vector.tensor_copy(identb[:], identf[:])
            nc.vector.memset(onesb[:], 1.0)

            # ---- phase A: x_sel transposes (pipelined with the xtp DMA
            # stream), then logits^T in the k-partitioned layout: the exp()
            # output IS attn^T (unnormalized; 1/den applied on host since
            # the whole V path is linear in attn). No max-subtraction:
            # logits*RSQ is O(1) and softmax is shift-invariant.
            with (
                tc.tile_pool(name="lgp", bufs=3, space="PSUM") as lgp,
                tc.tile_pool(name="trp", bufs=3, space="PSUM") as trp,
                tc.tile_pool(name="dnp", bufs=1, space="PSUM") as dnp,
            ):
                # GPSIMD cannot access PSUM -> only DVE + Activation here
                copy_fns = [nc.vector.tensor_copy, nc.scalar.copy,
                            nc.vector.tensor_copy, nc.scalar.copy]
                for dd in range(ND):
                    # transposes of d-chunk dd: 4 k-chunks share one psum
                    # tile as a single accumulation group (disjoint slices
                    # of the pending-zeroed bank); one strided copy each.
                    for kg in range(4):
                        pt = trp.tile([128, 4, 128], bf, tag="pt")
                        for j in range(4):
                            c = kg * 4 + j
                            nc.tensor.matmul(
                                pt[:, j, :],
                                xtp[:, dd, c * 128:(c + 1) * 128],
                                identb[:, :],
                                start=(j == 0), stop=(j == 3),
                                is_transpose=True)
                        # f32-bitcast: PSUM access must be 32-bit granular
                        copy_fns[kg](
                            xsp[:, kg * 4:kg * 4 + 4,
                                dd * 128:(dd + 1) * 128].bitcast(dt.float32),
                            pt[:].bitcast(dt.float32))

                # logits^T[k, h] accumulated per k-chunk; exp straight out
                # of PSUM into bf16 attnT (Activation engine).
                for c in range(NK):
                    plg = lgp.tile([128, H], dt.float32, tag="plg")
                    for dd in range(ND):
                        nc.tensor.matmul(
                            plg[:], xtp[:, dd, c * 128:(c + 1) * 128],
                            qhdt[:, dd, :],
                            start=(dd == 0), stop=(dd == ND - 1))
                    nc.scalar.activation(attnT[:, c, :], plg[:],
                                         mybir.ActivationFunctionType.Exp,
                                         scale=RSQ)

                # den[h] = sum_k attnT[k, h] via ones-matmul
                pden = dnp.tile([1, H], dt.float32)
                for c in range(NK):
                    nc.tensor.matmul(pden[:], onesb[:], attnT[:, c, :],
                                     start=(c == 0), stop=(c == NK - 1))
                nc.vector.tensor_copy(den1[:], pden[:])
                nc.scalar.dma_start(deno, den1[:])

            # ---- phase B: m^T, mv^T, o^T — tiny-N matmuls batched into
            # few PSUM groups (disjoint slices, one start/stop pair, one
            # copy) to keep semaphore/copy overhead off the critical path.
            with tc.tile_pool(name="smp", bufs=2, space="PSUM") as smp:
                # m^T: 4 d-chunks share one psum tile/group; mv^T
                # accumulates in a single 4-slice group as mT lands.
                pmv = smp.tile([128, NL, H], dt.float32, tag="mv", bufs=1)
                for dg in range(4):
                    mt = smp.tile([128, 4, H], dt.float32, tag="mt")
                    for j in range(4):
                        dd = dg * 4 + j
                        for c in range(NK):
                            nc.tensor.matmul(
                                mt[:, j, :],
                                xsp[:, c, dd * 128:(dd + 1) * 128],
                                attnT[:, c, :],
                                start=(j == 0 and c == 0),
                                stop=(j == 3 and c == NK - 1))
                    nc.vector.tensor_copy(
                        mT[:, dg * 4:dg * 4 + 4, :], mt[:])
                    for j in range(4):
                        dd = dg * 4 + j
                        for lc in range(NL):
                            nc.tensor.matmul(
                                pmv[:, lc, :],
                                wvdt[:, dd, lc * 128:(lc + 1) * 128],
                                mT[:, dd, :],
                                start=(dg == 0 and j == 0 and lc == 0),
                                stop=(dg == 3 and j == 3 and lc == NL - 1))
                nc.vector.tensor_copy(mvT[:], pmv[:])

                # o^T: N=1 moving (column h of mv^T) -> all 16 head
                # outputs land in one psum tile/group, one copy out.
                # lc-outer so only the last Wv_up chunk's 16 matmuls wait
                # on the final weight DMA.
                po = smp.tile([128, H], dt.float32, tag="po", bufs=1)
                for lc in range(NL):
                    for h in range(H):
                        nc.tensor.matmul(
                            po[:, h:h + 1],
                            wvut[:, lc, h * 128:(h + 1) * 128],
                            mvT[:, lc, h:h + 1],
                            start=(lc == 0 and h == 0),
                            stop=(lc == NL - 1 and h == H - 1))
                nc.vector.tensor_copy(oT[:], po[:])
            nc.sync.dma_start(oTo, oT[:])
    nc.compile()
    return nc


# ---------------------------------------------------------------- launch 3
def _build_l3():
    """Out-projection, tensor-parallel over cores: core c computes
    out[:, c*256:(c+1)*256] = o_all @ Wout[:, c*256:(c+1)*256] for ALL
    batches (o_all is only 32 KiB; each core loads 1/8th of Wout)."""
    nc = bacc.Bacc("TRN2", target_bir_lowering=False, debug=False,
                   num_devices=NCORES)
    bf = dt.bfloat16
    DOB = D // NCORES          # 256 output columns per core
    # o_all pre-packed on host to [p, (c b)] for contiguous descriptors
    oaP = nc.dram_tensor("oaP", [128, ND * B], bf, kind="ExternalInput").ap()
    wob = nc.dram_tensor("wob", [D, DOB], bf, kind="ExternalInput").ap()
    op = nc.dram_tensor("op", [B, DOB], dt.float32, kind="ExternalOutput").ap()

    with tile.TileContext(nc) as tc:
        with (
            tc.tile_pool(name="l3p", bufs=1) as l3p,
            tc.tile_pool(name="l3ps", bufs=1, space="PSUM") as l3ps,
        ):
            oat = l3p.tile([128, ND, B], bf)
            wot = l3p.tile([128, ND, DOB], bf)
            # o_all in one tiny DMA, Wout block 4-way split so the matmul
            # chain trails the stream and only the last quarter's matmuls
            # wait on the final transfer
            nc.sync.dma_start(
                oat[:], oaP.rearrange("p (c b) -> p c b", b=B))
            NS = ND // 4
            for s in range(4):
                cs = slice(s * NS, (s + 1) * NS)
                nc.sync.dma_start(
                    wot[:, cs, :],
                    wob[s * NS * 128:(s + 1) * NS * 128, :].rearrange(
                        "(c p) d -> p c d", p=128))
            pp = l3ps.tile([B, DOB], dt.float32)
            for c in range(ND):
                nc.tensor.matmul(pp[:], oat[:, c, :], wot[:, c, :],
                                 start=(c == 0), stop=(c == ND - 1))
            ops = l3p.tile([B, DOB], dt.float32)
            nc.vector.tensor_copy(ops[:], pp[:])
            nc.sync.dma_start(op, ops[:])
    nc.compile()
    return nc


# ---------------------------------------------------------------- timing
def time_launch(nc, in_maps, iters=20):
    """Measure per-execution HW time of a compiled launch: build the sharded
    PJRT executable once, keep inputs device-resident, pipeline `iters`
    executions and average."""
    import time as _time
    import jax
    from jax.sharding import Mesh, PartitionSpec, NamedSharding
    from jax.experimental.shard_map import shard_map
    from concourse import bass2jax

    bass2jax.install_neuronx_cc_hook()
    pname = nc.partition_id_tensor.name if nc.partition_id_tensor else None
    in_names, out_names, out_avals = [], [], []
    for alloc in nc.m.functions[0].allocations:
        if not isinstance(alloc, mybir.MemoryLocationSet):
            continue
        name = alloc.memorylocations[0].name
        if alloc.kind == "ExternalInput":
            if name != pname:
                in_names.append(name)
        elif alloc.kind == "ExternalOutput":
            out_names.append(name)
            out_avals.append(jax.core.ShapedArray(
                tuple(alloc.tensor_shape), mybir.dt.np(alloc.dtype)))
    n_params = len(in_names)
    all_in = in_names + out_names
    if pname is not None:
        all_in = all_in + [pname]
    donate = tuple(range(n_params, n_params + len(out_names)))

    def _body(*args):
        operands = list(args)
        if pname is not None:
            operands.append(bass2jax.partition_id_tensor())
        outs = bass2jax._bass_exec_p.bind(
            *operands, out_avals=tuple(out_avals), in_names=tuple(all_in),
            out_names=tuple(out_names), lowering_input_output_aliases=(),
            sim_require_finite=True, sim_require_nnan=True, nc=nc)
        return tuple(outs)

    n = len(in_maps)
    devices = jax.devices()[:n]
    mesh = Mesh(np.asarray(devices), ("core",))
    fn = jax.jit(
        shard_map(_body, mesh=mesh,
                  in_specs=(PartitionSpec("core"),) * (n_params + len(out_names)),
                  out_specs=(PartitionSpec("core"),) * len(out_names),
                  check_rep=False),
        donate_argnums=donate, keep_unused=True)
    sh = NamedSharding(mesh, PartitionSpec("core"))
    concat_in = [
        jax.device_put(
            np.concatenate([np.asarray(m[name]) for m in in_maps], axis=0), sh)
        for name in in_names]

    def zeros():
        return [jax.device_put(
            np.zeros((n * av.shape[0], *av.shape[1:]), av.dtype), sh)
            for av in out_avals]

    out = fn(*concat_in, *zeros())
    jax.block_until_ready(out)
    zs = [zeros() for _ in range(iters)]
    jax.block_until_ready(zs)
    t0 = _time.perf_counter()
    outs = [fn(*concat_in, *z) for z in zs]
    jax.block_until_ready(outs)
    t1 = _time.perf_counter()
    return (t1 - t0) / iters * 1e9


def model_time(nc):
    """Cost-model (TimelineSim) estimate in ns for one core."""
    from concourse.timeline_sim import TimelineSim
    return TimelineSim(nc).simulate()


def _run_spmd_retry(nc, in_maps, cores, trace=False):
    """One retry: a previously crashed process can leave the device in a
    transient NRT_EXEC_UNIT_UNRECOVERABLE state that clears on re-run."""
    try:
        return run_bass_kernel_spmd(nc, in_maps, cores, trace=trace)
    except Exception:
        import time as _t
        _t.sleep(2.0)
        return run_bass_kernel_spmd(nc, in_maps, cores, trace=trace)


def _q8j(a):
    import jax.numpy as jnp
    return jnp.asarray(a).astype(jnp.float8_e4m3fn).astype(jnp.float32)


def kernel(**inputs):
    import jax
    import jax.numpy as jnp
    cpu = jax.devices("cpu")[0]

    x = np.ascontiguousarray(np.asarray(inputs["x"], dtype=np.float32))
    Wq = np.asarray(inputs["Wq"], dtype=np.float32)
    bq = np.asarray(inputs["bq"], dtype=np.float32)
    Wkv_down = np.asarray(inputs["Wkv_down"], dtype=np.float32)
    bkv_down = np.asarray(inputs["bkv_down"], dtype=np.float32)
    Wq_down = np.asarray(inputs["Wq_down"], dtype=np.float32)
    bq_down = np.asarray(inputs["bq_down"], dtype=np.float32)
    Wkv_up = np.asarray(inputs["Wkv_up"], dtype=np.float32)
    bkv_up = np.asarray(inputs["bkv_up"], dtype=np.float32)
    Wout = np.asarray(inputs["Wout"], dtype=np.float32)
    bout = np.asarray(inputs["bout"], dtype=np.float32)
    k = int(np.asarray(inputs["top_k"]))
    assert k == TOPK, f"kernel hardcoded for top_k={TOPK}, got {k}"

    if "l1" not in _STATE:
        _STATE["l1"] = _build_l1()
    if "l2" not in _STATE:
        _STATE["l2"] = _build_l2()
    if "l3" not in _STATE:
        _STATE["l3"] = _build_l3()

    trace = False  # NTFF profiling hook unavailable under this axon client

    q_last = x[:, -1, :]                                   # [B, D]
    with jax.default_device(cpu):
        # bit-exact replication of the reference's fp8 indexer query + q
        q_idx = np.asarray(_q8j(q_last) @ _q8j(Wq_down) + _q8j(bq_down))
        q = np.asarray(jnp.asarray(q_last) @ jnp.asarray(Wq)) + bq

    Wdk = Wkv_down[:, :L]                                  # [D, L]

    # ---------------- launch 1: noisy full-S scores s = x . (Wdk @ q_idx)
    v = q_idx @ Wdk.T                                      # [B, D] f32
    in1 = []
    for c in range(NCORES):
        v8m = np.zeros((128, ND * 32), np.float32)
        v8m[:, np.arange(ND) * 32] = v[c].reshape(ND, 128).T
        in1.append({
            "x8": np.ascontiguousarray(x[c].T).astype(F8),
            "v8": v8m.astype(F8),
        })
    r1 = _run_spmd_retry(_STATE["l1"], in1, list(range(NCORES)), trace=trace)
    LAST_EXEC["l1"] = r1
    s_noisy = np.stack([np.asarray(r1.results[c]["scores"], np.float32)[0]
                        for c in range(NCORES)])           # [B, S]
    s_noisy = s_noisy + (q_idx @ bkv_down[:L])[:, None]    # K_down bias term

    # ---------------- host: exact top-k set via band rescore (bit-exact)
    sel_all = []
    with jax.default_device(cpu):
        jWdk = jnp.asarray(Wdk)
        jbkd = jnp.asarray(bkv_down[:L])
        for b in range(B):
            order = np.argsort(-np.maximum(s_noisy[b], 0.0), kind="stable")
            certain = order[:k - MARGIN]
            band = order[k - MARGIN:k + MARGIN]
            Kb = jnp.asarray(x[b][band]) @ jWdk + jbkd
            sb = np.asarray(jnp.einsum(
                "l,sl->s", jnp.asarray(q_idx[b]),
                Kb.astype(jnp.float8_e4m3fn).astype(jnp.float32)))
            sb = np.maximum(sb, 0.0)
            # ties broken by token index (reference top_k is index-stable)
            pick = band[np.lexsort((band, -sb))[:k - len(certain)]]
            sel_all.append(np.concatenate([certain, pick]))

    # ---------------- launch 2: attention over the selected set -> o
    Wk_up = Wkv_up[:, :D]                                  # [L, D]
    Wv_up = Wkv_up[:, D:]                                  # [L, D]
    wvd8 = np.ascontiguousarray(Wkv_down[:, L:]).astype(BF16)
    wvup8 = np.ascontiguousarray(Wv_up).astype(BF16)
    Wk3 = Wk_up.reshape(L, H, DH)
    in2 = []
    for c in range(NCORES):
        xs = x[c][sel_all[c]]                              # [k, D]
        qh = np.einsum("hd,lhd->hl", q[c].reshape(H, DH), Wk3)   # [H, L]
        qhd = qh @ Wdk.T                                   # [H, D]
        # pack [D, H] -> [128, (chunk, h)] partition-major for the DMA
        qhdp = np.ascontiguousarray(
            qhd.T.reshape(ND, 128, H).transpose(1, 0, 2).reshape(128, ND * H))
        in2.append({
            "xselT": np.ascontiguousarray(xs.T).astype(BF16),
            "qhdP": qhdp.astype(BF16),
            "wvd": wvd8,
            "wvup": wvup8,
        })
    r2 = _run_spmd_retry(_STATE["l2"], in2, list(range(NCORES)), trace=trace)
    LAST_EXEC["l2"] = r2
    # o_dev[b] = flattened [H, DH] head outputs; device attn is
    # unnormalized, so divide by the exported softmax denominators here
    # (the V path is linear in attn).
    o_all = np.stack([
        (np.asarray(r2.results[c]["oTo"], np.float32)
         / np.asarray(r2.results[c]["deno"], np.float32)).T.reshape(D)
        for c in range(NCORES)])                           # [B, D]

    # ---------------- launch 3: out = o_all @ Wout (column-split over cores)
    DOB = D // NCORES
    oap8 = np.ascontiguousarray(
        o_all.T.reshape(ND, 128, B).transpose(1, 0, 2).reshape(
            128, ND * B)).astype(BF16)                     # [128, (c b)]
    wout8 = np.ascontiguousarray(Wout).astype(BF16)
    in3 = [{"oaP": oap8,
            "wob": np.ascontiguousarray(
                wout8[:, c * DOB:(c + 1) * DOB])}
           for c in range(NCORES)]
    r3 = _run_spmd_retry(_STATE["l3"], in3, list(range(NCORES)), trace=trace)
    LAST_EXEC["l3"] = r3
    out = np.concatenate(
        [np.asarray(r3.results[c]["op"], np.float32) for c in range(NCORES)],
        axis=1)                                            # [B, D]

    # V-path + out biases folded via sum(attn)==1:
    #   o_full = o_dev + cv,  out = o_full @ Wout + bout = o_dev @ Wout + bout'
    cv = bkv_down[L:] @ Wv_up + bkv_up[D:]                 # [D]
    bout2 = cv @ Wout + bout                               # [D]
    return (out + bout2[None, :]).astype(np.float32)
